# revision 11
# baseline (speedup 1.0000x reference)
"""Depth-to-space (CRD order) kernel for Trainium2, 8 NeuronCores.

in:  (32, 9, 512, 512) f32, channel c = r*3+s encodes (row_off, col_off)
out: (32, 1, 1536, 1536) f32 with out[b,0,3i+r,3j+s] = in[b,3r+s,i,j]

Sharding: data-parallel over batch, 4 batches per core, no communication.
Per core per (batch, 128-row chunk):
  - one DMA-in  of x[b, :, i0:i0+128, :] -> SBUF [128, 9*512]   (2.25 MB)
  - three strided-AP copies (one per output-row offset r) interleaving
    channels 3r+0..3r+2 into contiguous output rows
  - one DMA-out of [128, 3*1536] -> y rows 3*i0 .. 3*i0+384      (2.25 MB,
    fully contiguous in DRAM)
"""

import sys

import numpy as np

_B, _C, _H, _W = 32, 9, 512, 512
_K = 3
_NCORES = 8
_BLOC = _B // _NCORES  # 4

_PROG = None


def _ensure_path():
    try:
        import concourse.bass  # noqa: F401
    except ImportError:
        sys.path.insert(0, "/opt/trn_rl_repo")


def _build():
    import concourse.bacc as bacc
    import concourse.mybir as mybir
    from concourse import tile

    f32 = mybir.dt.float32
    nc = bacc.Bacc(None)
    x = nc.declare_dram_parameter("x", [_BLOC, _C, _H, _W], f32, isOutput=False)
    y = nc.declare_dram_parameter("y", [_BLOC, _K * _H, _K * _W], f32, isOutput=True)

    P = 128
    KW = _K * _W  # 1536

    with tile.TileContext(nc) as tc:
        with (
            tc.tile_pool(name="tin", bufs=3) as pin,
            tc.tile_pool(name="tout", bufs=3) as pout,
        ):
            su = 0
            for b in range(_BLOC):
                for i0 in range(0, _H, P):
                    # output rows 3*i0 .. 3*i0+384, grouped by row offset r
                    dst = y[b, _K * i0 : _K * (i0 + P), :].rearrange(
                        "(p r) w -> r p w", r=_K
                    )
                    for r in range(_K):
                        # dedicated HWDGE rings: SP carries loads, ACT stores;
                        # mixing them on one ring lets a not-yet-ready store
                        # block ready loads behind it (FIFO per ring)
                        ld_eng, st_eng = nc.sync, nc.scalar
                        su += 1
                        # copy r consumes exactly channels 3r..3r+2
                        tin = pin.tile([P, KW], f32)
                        ld_eng.dma_start(
                            out=tin[:].rearrange("p (s j) -> p s j", s=_K),
                            in_=x[b, _K * r : _K * (r + 1), i0 : i0 + P, :].rearrange(
                                "s p j -> p s j"
                            ),
                        )
                        # out[p, 3j+s] = in[p, s*512+j]
                        tout = pout.tile([P, KW], f32)
                        nc.vector.tensor_copy(
                            out=tout[:].rearrange("p (j s) -> p j s", s=_K),
                            in_=tin[:].rearrange("p (s j) -> p j s", s=_K),
                        )
                        st_eng.dma_start(out=dst[r], in_=tout[:])
    return nc


def _run(x_full, trace=False, **spmd_kwargs):
    """x_full: (32, 9, 512, 512) f32 ndarray. Returns (out, BassKernelResults)."""
    global _PROG
    _ensure_path()
    from concourse.bass_utils import run_bass_kernel_spmd

    if _PROG is None:
        _PROG = _build()
        if not _PROG.is_finalized():
            _PROG.finalize()
    in_maps = [
        {"x": np.ascontiguousarray(x_full[i * _BLOC : (i + 1) * _BLOC])}
        for i in range(_NCORES)
    ]
    res = run_bass_kernel_spmd(
        _PROG, in_maps, core_ids=list(range(_NCORES)), trace=trace, **spmd_kwargs
    )
    out = np.concatenate([np.asarray(r["y"]) for r in res.results], axis=0)
    return out.reshape(_B, 1, _K * _H, _K * _W), res


def kernel(**inputs):
    x = np.ascontiguousarray(np.asarray(inputs["inputs"], dtype=np.float32))
    k = int(np.asarray(inputs.get("kernel_size", _K)))
    assert k == _K, f"kernel hardcodes kernel_size=3, got {k}"
    assert x.shape == (_B, _C, _H, _W), x.shape
    out, _ = _run(x)
    return out


# revision 12
# speedup vs baseline: 1.0050x; 1.0050x over previous
"""Depth-to-space (CRD order) kernel for Trainium2, 8 NeuronCores.

in:  (32, 9, 512, 512) f32, channel c = r*3+s encodes (row_off, col_off)
out: (32, 1, 1536, 1536) f32 with out[b,0,3i+r,3j+s] = in[b,3r+s,i,j]

Sharding: data-parallel over batch, 4 batches per core, no communication.
Per core per (batch, 128-row chunk):
  - one DMA-in  of x[b, :, i0:i0+128, :] -> SBUF [128, 9*512]   (2.25 MB)
  - three strided-AP copies (one per output-row offset r) interleaving
    channels 3r+0..3r+2 into contiguous output rows
  - one DMA-out of [128, 3*1536] -> y rows 3*i0 .. 3*i0+384      (2.25 MB,
    fully contiguous in DRAM)
"""

import sys

import numpy as np

_B, _C, _H, _W = 32, 9, 512, 512
_K = 3
_NCORES = 8
_BLOC = _B // _NCORES  # 4

_PROG = None


def _ensure_path():
    try:
        import concourse.bass  # noqa: F401
    except ImportError:
        sys.path.insert(0, "/opt/trn_rl_repo")


def _build():
    import concourse.bacc as bacc
    import concourse.mybir as mybir
    from concourse import tile

    f32 = mybir.dt.float32
    nc = bacc.Bacc(None)
    x = nc.declare_dram_parameter("x", [_BLOC, _C, _H, _W], f32, isOutput=False)
    y = nc.declare_dram_parameter("y", [_BLOC, _K * _H, _K * _W], f32, isOutput=True)

    P = 128
    KW = _K * _W  # 1536

    with tile.TileContext(nc) as tc:
        with (
            tc.tile_pool(name="tin", bufs=3) as pin,
            tc.tile_pool(name="tout", bufs=3) as pout,
        ):
            su = 0
            for b in range(_BLOC):
                for i0 in range(0, _H, P):
                    # output rows 3*i0 .. 3*i0+384, grouped by row offset r
                    dst = y[b, _K * i0 : _K * (i0 + P), :].rearrange(
                        "(p r) w -> r p w", r=_K
                    )
                    for r in range(_K):
                        # dedicated HWDGE rings: SP carries loads, ACT stores;
                        # mixing them on one ring lets a not-yet-ready store
                        # block ready loads behind it (FIFO per ring). The
                        # edges are safe exceptions: first loads ride the
                        # still-idle store ring, last stores the drained load
                        # ring (no younger work queues behind them there).
                        ld_eng = nc.scalar if su < 2 else nc.sync
                        st_eng = nc.sync if su >= 46 else nc.scalar
                        su += 1
                        # copy r consumes exactly channels 3r..3r+2
                        tin = pin.tile([P, KW], f32)
                        ld_eng.dma_start(
                            out=tin[:].rearrange("p (s j) -> p s j", s=_K),
                            in_=x[b, _K * r : _K * (r + 1), i0 : i0 + P, :].rearrange(
                                "s p j -> p s j"
                            ),
                        )
                        # out[p, 3j+s] = in[p, s*512+j]
                        tout = pout.tile([P, KW], f32)
                        nc.vector.tensor_copy(
                            out=tout[:].rearrange("p (j s) -> p j s", s=_K),
                            in_=tin[:].rearrange("p (s j) -> p j s", s=_K),
                        )
                        st_eng.dma_start(out=dst[r], in_=tout[:])
    return nc


def _run(x_full, trace=False, **spmd_kwargs):
    """x_full: (32, 9, 512, 512) f32 ndarray. Returns (out, BassKernelResults)."""
    global _PROG
    _ensure_path()
    from concourse.bass_utils import run_bass_kernel_spmd

    if _PROG is None:
        _PROG = _build()
        if not _PROG.is_finalized():
            _PROG.finalize()
    in_maps = [
        {"x": np.ascontiguousarray(x_full[i * _BLOC : (i + 1) * _BLOC])}
        for i in range(_NCORES)
    ]
    res = run_bass_kernel_spmd(
        _PROG, in_maps, core_ids=list(range(_NCORES)), trace=trace, **spmd_kwargs
    )
    out = np.concatenate([np.asarray(r["y"]) for r in res.results], axis=0)
    return out.reshape(_B, 1, _K * _H, _K * _W), res


def kernel(**inputs):
    x = np.ascontiguousarray(np.asarray(inputs["inputs"], dtype=np.float32))
    k = int(np.asarray(inputs.get("kernel_size", _K)))
    assert k == _K, f"kernel hardcodes kernel_size=3, got {k}"
    assert x.shape == (_B, _C, _H, _W), x.shape
    out, _ = _run(x)
    return out
